# revision 39
# baseline (speedup 1.0000x reference)
"""DeltaDequantization Trainium2 kernel (8-core SPMD, pure data parallel over batch).

Math (per batch element b, chunks c of 32 steps):
    scale_c = (1/32) * sum_{s,n} x[b,c,s,n] * cs[n]          (independent of carry!)
    S_c     = prod_{c'<c} scale_c'          (exclusive cumprod)
    y[b,t]  = sum_n x[b,t,n] * qb[n]
    m_c     = (1/32) * sum_{s in c} y[b,t]
    pred_c  = sum_{c'<c} S_c' * m_c'        (exclusive cumsum)
    out[b,t]= pred_c(t) + S_c(t) * y[b,t]

Kernel: load x naturally [128b, (t,n)], PE-transpose 128x128 blocks to get
(t',n) on partitions, one [128,8] matmul computes y and w=x@cs for 4 t's at a
time, PE-transpose back to [b,t], tensor_tensor_scan for the 64-step
recurrences, affine, store.

Pipelining notes (measured on HW):
- The 16 SWDGE DMA engines run 100% busy during the x stream; per-engine
  HBM-side rate peaks at ~25 B/ns with 8 KiB DRAM rows (quarter-span grain).
  Larger (16K) and smaller (4K) rows are slower.  The stream, not HBM, is
  the bottleneck, so total DMA bytes set an ~85 us floor for the window.
- Engine queues execute in (priority = program) order with head-of-line
  blocking, so span sp's post-matmul stages are EMITTED inside span sp+1's
  pb loop at points where their dependencies are already satisfied.
- The last span's post-processing runs at half-span granularity so only the
  final half-span's chain sits on the serial tail.
"""

import numpy as np
from contextlib import ExitStack

import concourse.bass as bass
import concourse.bacc as bacc
import concourse.tile as tile
from concourse import mybir
from concourse.bass_utils import run_bass_kernel_spmd
from concourse.masks import make_identity

F32 = mybir.dt.float32
BF16 = mybir.dt.bfloat16

B, T, NB = 1024, 2048, 32
NCORES = 8
BS = B // NCORES          # 128 batch rows per core = full partition dim
ADAPT = 32
C = T // ADAPT            # 64 chunks
SPAN_T = 256              # timesteps per pipelined span
NSPAN = T // SPAN_T       # 8
SPAN_F = SPAN_T * NB      # 8192 f32 elements per partition per span
CPS = SPAN_T // ADAPT     # 8 chunks per span

_cached_nc = None


def build_kernel():
    nc = bacc.Bacc("TRN2", target_bir_lowering=False, debug=False)

    x_ext = nc.dram_tensor("x", [BS, T * NB], F32, kind="ExternalInput")
    qb_ext = nc.dram_tensor("quant_bins", [NB, 1], F32, kind="ExternalInput")
    cs_ext = nc.dram_tensor("change_scales", [NB, 1], F32, kind="ExternalInput")
    out_ext = nc.dram_tensor("out", [BS, T], F32, kind="ExternalOutput")

    with tile.TileContext(nc) as tc:
        with (
            tc.tile_pool(name="consts", bufs=1) as consts,
            tc.tile_pool(name="xpool", bufs=10) as xpool,
            tc.tile_pool(name="xtpool", bufs=3) as xtpool,
            tc.tile_pool(name="midpool", bufs=2) as midpool,
            tc.tile_pool(name="accpool", bufs=1) as accpool,
            tc.tile_pool(name="smallpool", bufs=1) as smallpool,
            tc.tile_pool(name="ps_t", bufs=4, space="PSUM") as ps_t,
            tc.tile_pool(name="ps_yw", bufs=2, space="PSUM") as ps_yw,
            tc.tile_pool(name="ps_b", bufs=2, space="PSUM") as ps_b,
        ):
            # Issue the first x cast-loads before anything else touches the
            # GpSimd queue so HBM streaming starts at the preamble's end.
            # Load plan (in pb units of 32 t = 4 KiB f32 rows):
            # quarter-spans (8 KiB rows, the per-engine sweet spot)
            # throughout, split into eighth-spans at the end so the tail's
            # transposes start while the final bytes are in flight.
            PBF = SPAN_F // 8          # f32 elems per pb per partition
            plan = [4] + [2] * 29 + [1, 1]
            assert sum(plan) == NSPAN * 8
            load_pb0 = []
            acc = 0
            for cnt in plan:
                load_pb0.append(acc)
                acc += cnt
            pb2load = {}
            for li, (p0, cnt) in enumerate(zip(load_pb0, plan)):
                for w in range(cnt):
                    pb2load[p0 + w] = (li, w)
            xq = []
            next_issue = [0]
            AHEAD_PB = 16

            def issue_one():
                li = next_issue[0]
                cnt, p0 = plan[li], load_pb0[li]
                x_h = xpool.tile([128, cnt * PBF], BF16)
                nc.gpsimd.dma_start(
                    out=x_h[:], in_=x_ext[:, p0 * PBF:(p0 + cnt) * PBF]
                )
                xq.append(x_h)
                next_issue[0] += 1

            def issue_ahead(global_pb):
                while (
                    next_issue[0] < len(plan)
                    and load_pb0[next_issue[0]] <= global_pb + AHEAD_PB
                ):
                    issue_one()

            issue_one()
            issue_one()

            ident = consts.tile([128, 128], F32)
            make_identity(nc, ident[:])
            ident_bf = consts.tile([128, 128], BF16)
            make_identity(nc, ident_bf[:])

            issue_ahead(-2)  # remaining loads covering pb 0..13

            # Four stationary matrices A32_q [128, 32], q = 0..3.
            # Column m = 16*j + 4*q + t''; A32_q[(t', n), m] = delta(t', t'') *
            # (qb[n] if j == 0 else cs[n]); zero columns for other q values.
            # Stage qb/cs via the Activation HWDGE queue: the Sync queue
            # carries one companion descriptor per SWDGE load.
            qbcs = consts.tile([128, 2], F32)
            for tp in range(4):
                nc.scalar.dma_start(out=qbcs[32 * tp:32 * tp + 32, 0:1], in_=qb_ext[:])
                nc.scalar.dma_start(out=qbcs[32 * tp:32 * tp + 32, 1:2], in_=cs_ext[:])
            A32 = []
            for q in range(4):
                Aq = consts.tile([128, 32], BF16, tag=f"A32_{q}")
                nc.vector.memset(Aq[:], 0.0)
                for tp in range(4):
                    sl = slice(32 * tp, 32 * tp + 32)
                    nc.vector.tensor_copy(
                        out=Aq[sl, 4 * q + tp:4 * q + tp + 1], in_=qbcs[sl, 0:1]
                    )
                    nc.vector.tensor_copy(
                        out=Aq[sl, 16 + 4 * q + tp:16 + 4 * q + tp + 1], in_=qbcs[sl, 1:2]
                    )
                A32.append(Aq)

            # Persistent per-core accumulators
            y_sb = accpool.tile([128, T], F32)
            w_sb = accpool.tile([128, T], F32)
            out_sb = accpool.tile([128, T], F32)

            m_term = smallpool.tile([128, C], F32)
            p_sc = smallpool.tile([128, C], F32)
            S_exc = smallpool.tile([128, C + 1], F32)
            pred = smallpool.tile([128, C + 1], F32)
            tau = smallpool.tile([128, C], F32)
            nc.vector.memset(S_exc[:, 0:1], 1.0)
            nc.vector.memset(pred[:, 0:1], 0.0)

            yw_mid_of = {}
            ps2_of = {}

            def emit_ywmid(sp, ps_y):
                yw_mid = midpool.tile([128, 512], F32)
                nc.scalar.copy(out=yw_mid[:], in_=ps_y[:])
                yw_mid_of[sp] = yw_mid

            def emit_backT(sp):
                yw_mid = yw_mid_of.pop(sp)
                ps2 = ps_b.tile([128, 512], F32, tag="ps2")
                for blk2 in range(4):
                    nc.tensor.transpose(
                        ps2[:, blk2 * 128:(blk2 + 1) * 128],
                        yw_mid[:, blk2 * 128:(blk2 + 1) * 128],
                        ident[:],
                    )
                ps2_of[sp] = ps2

            def emit_ywcopies_reduce(sp):
                # ps2 free index = 128*blk + 32*g4 + 16*j + 4*q + t''
                # t(within span) = 64*g4 + 16*q + 4*blk + t''
                ps2 = ps2_of.pop(sp)
                ps2v = ps2[:].rearrange(
                    "p (b g j q t) -> p g q b j t", b=4, g=4, j=2, q=4, t=4
                )
                yspan = y_sb[:, sp * SPAN_T:(sp + 1) * SPAN_T].rearrange(
                    "p (g q b t) -> p g q b t", g=4, q=4, b=4, t=4
                )
                wspan = w_sb[:, sp * SPAN_T:(sp + 1) * SPAN_T].rearrange(
                    "p (g q b t) -> p g q b t", g=4, q=4, b=4, t=4
                )
                nc.vector.tensor_copy(out=yspan, in_=ps2v[:, :, :, :, 0:1, :].squeeze(4))
                nc.vector.tensor_copy(out=wspan, in_=ps2v[:, :, :, :, 1:2, :].squeeze(4))

                csl = slice(sp * CPS, (sp + 1) * CPS)
                nc.vector.tensor_reduce(
                    out=m_term[:, csl],
                    in_=y_sb[:, sp * SPAN_T:(sp + 1) * SPAN_T].rearrange(
                        "p (c s) -> p c s", c=CPS, s=ADAPT
                    ),
                    axis=mybir.AxisListType.X,
                    op=mybir.AluOpType.add,
                )
                nc.vector.tensor_reduce(
                    out=p_sc[:, csl],
                    in_=w_sb[:, sp * SPAN_T:(sp + 1) * SPAN_T].rearrange(
                        "p (c s) -> p c s", c=CPS, s=ADAPT
                    ),
                    axis=mybir.AxisListType.X,
                    op=mybir.AluOpType.add,
                )

            def emit_scans(c_lo, c_hi, eng=None):
                eng = eng or nc.vector
                sl = slice(c_lo, c_hi)
                eng.tensor_scalar_mul(m_term[:, sl], m_term[:, sl], 1.0 / ADAPT)
                eng.tensor_scalar_mul(p_sc[:, sl], p_sc[:, sl], 1.0 / ADAPT)
                eng.tensor_tensor_scan(
                    out=S_exc[:, c_lo + 1:c_hi + 1],
                    data0=p_sc[:, sl],
                    data1=p_sc[:, sl],
                    initial=S_exc[:, c_lo:c_lo + 1],
                    op0=mybir.AluOpType.mult,
                    op1=mybir.AluOpType.bypass,
                )
                eng.tensor_mul(out=tau[:, sl], in0=S_exc[:, sl], in1=m_term[:, sl])
                eng.tensor_tensor_scan(
                    out=pred[:, c_lo + 1:c_hi + 1],
                    data0=tau[:, sl],
                    data1=tau[:, sl],
                    initial=pred[:, c_lo:c_lo + 1],
                    op0=mybir.AluOpType.add,
                    op1=mybir.AluOpType.bypass,
                )

            def emit_affine_store(c_lo, c_hi, eng=None, store=True):
                eng = eng or nc.vector
                sl = slice(c_lo, c_hi)
                nch = c_hi - c_lo
                t_lo, t_hi = c_lo * ADAPT, c_hi * ADAPT
                y3 = y_sb[:, t_lo:t_hi].rearrange("p (c s) -> p c s", c=nch, s=ADAPT)
                o3 = out_sb[:, t_lo:t_hi].rearrange("p (c s) -> p c s", c=nch, s=ADAPT)
                S_b = S_exc[:, sl].unsqueeze(2).broadcast_to([128, nch, ADAPT])
                pred_b = pred[:, sl].unsqueeze(2).broadcast_to([128, nch, ADAPT])
                eng.tensor_mul(out=o3, in0=y3, in1=S_b)
                eng.tensor_add(out=o3, in0=o3, in1=pred_b)
                if store:
                    nc.sync.dma_start(
                        out=out_ext[:, t_lo:t_hi], in_=out_sb[:, t_lo:t_hi]
                    )

            # ---- half-span post stages (last span only) ----
            ps2h_of = {}

            def emit_ywmid_h(sp, h, src):
                yw_mid = midpool.tile([128, 512], F32)
                nc.scalar.copy(out=yw_mid[64 * h:64 * h + 64, :],
                               in_=src[64 * h:64 * h + 64, :])
                yw_mid_of[(sp, h)] = yw_mid

            def emit_backT_h(sp, h):
                yw_mid = yw_mid_of.pop((sp, h))
                # Full-width tile: shares the ps_b pool's single tag/slots.
                ps2 = ps_b.tile([128, 512], F32, tag="ps2")
                for blk2 in range(4):
                    nc.tensor.transpose(
                        ps2[:, blk2 * 64:(blk2 + 1) * 64],
                        yw_mid[64 * h:64 * h + 64, blk2 * 128:(blk2 + 1) * 128],
                        ident[64 * h:64 * h + 64, 64 * h:64 * h + 64],
                    )
                ps2h_of[(sp, h)] = ps2

            def emit_ywcopies_reduce_h(sp, h):
                # ps2 free index = 64*blk + 32*gl + 16*j + 4*q + t''  (gl = g4-2h)
                # t(within span) = 128*h + 64*gl + 16*q + 4*blk + t''
                ps2 = ps2h_of.pop((sp, h))
                ps2v = ps2[:, 0:256].rearrange(
                    "p (b g j q t) -> p g q b j t", b=4, g=2, j=2, q=4, t=4
                )
                t0 = sp * SPAN_T + 128 * h
                yspan = y_sb[:, t0:t0 + 128].rearrange(
                    "p (g q b t) -> p g q b t", g=2, q=4, b=4, t=4
                )
                wspan = w_sb[:, t0:t0 + 128].rearrange(
                    "p (g q b t) -> p g q b t", g=2, q=4, b=4, t=4
                )
                nc.vector.tensor_copy(out=yspan, in_=ps2v[:, :, :, :, 0:1, :].squeeze(4))
                nc.vector.tensor_copy(out=wspan, in_=ps2v[:, :, :, :, 1:2, :].squeeze(4))

                c0 = sp * CPS + 4 * h
                csl = slice(c0, c0 + 4)
                nc.vector.tensor_reduce(
                    out=m_term[:, csl],
                    in_=y_sb[:, t0:t0 + 128].rearrange(
                        "p (c s) -> p c s", c=4, s=ADAPT
                    ),
                    axis=mybir.AxisListType.X,
                    op=mybir.AluOpType.add,
                )
                nc.vector.tensor_reduce(
                    out=p_sc[:, csl],
                    in_=w_sb[:, t0:t0 + 128].rearrange(
                        "p (c s) -> p c s", c=4, s=ADAPT
                    ),
                    axis=mybir.AxisListType.X,
                    op=mybir.AluOpType.add,
                )

            LAST = NSPAN - 1
            for sp in range(NSPAN):
                xT_sp = xtpool.tile([128, SPAN_F], BF16)
                # y/w projection, interleaved with the transposes that feed it.
                # Group g = g4*4+q covers blocks 4g..4g+3 (t = 16g + 4blk + t'').
                # Strip g4 accumulates 4 matmuls into ps_y[32*g4 : 32*g4+32, :];
                # dense partition layout p = 32*g4 + 16*j + 4*q + t'', free=(blk,b).
                ps_y = ps_yw.tile([128, 512], F32, tag="psy")
                ps_y2 = None
                for pb in range(8):
                    # SWDGE cast-load f32 DRAM -> bf16 SBUF, planned grain
                    global_pb = sp * 8 + pb
                    issue_ahead(global_pb)
                    li, w = pb2load[global_pb]
                    x_h = xq[li]
                    if sp == LAST and pb == 4:
                        # Separate PSUM tile for strips 2-3 so the half-0
                        # yw_mid read doesn't serialize these matmuls
                        # (tile-granular write-after-read hazard).
                        ps_y2 = ps_yw.tile([128, 512], F32, tag="psy")
                    pst = ps_t.tile([128, 1024], BF16)
                    for k in range(8):
                        blk_l = w * 8 + k
                        nc.tensor.transpose(
                            pst[:, k * 128:(k + 1) * 128],
                            x_h[:, blk_l * 128:(blk_l + 1) * 128],
                            ident_bf[:],
                        )
                    dst = xT_sp[:, pb * 1024:(pb + 1) * 1024]
                    # Last span: keep the Scalar queue free for the yw_mid
                    # copies that gate the tail chain.
                    if pb % 2 == 0 or (sp == LAST and pb >= 5):
                        nc.vector.tensor_copy(out=dst, in_=pst[:])
                    else:
                        nc.scalar.copy(out=dst, in_=pst[:])
                    for gg in range(2):
                        g = pb * 2 + gg
                        g4, q = divmod(g, 4)
                        tgt = ps_y2 if (sp == LAST and g4 >= 2) else ps_y
                        nc.tensor.matmul(
                            tgt[32 * g4:32 * g4 + 32, :],
                            A32[q][:],
                            xT_sp[:, g * 512:(g + 1) * 512],
                            start=(q == 0),
                            stop=(q == 3),
                            tile_position=(0, 32 * g4),
                        )
                    # Previous span's post stages, placed where their deps are
                    # already met so they never head-of-line-block this span.
                    if sp > 0:
                        # In the last span, run the previous span's scan and
                        # affine one pb earlier so they clear the Vector
                        # queue before the tail's PSUM-drain copies arrive.
                        sh = 1 if sp == LAST else 0
                        if pb == 1:
                            emit_backT(sp - 1)
                        elif pb == 2:
                            emit_ywcopies_reduce(sp - 1)
                            if sh:
                                emit_scans((sp - 1) * CPS, sp * CPS)
                        elif pb == 3 - sh + 0 and pb >= 3 - sh and pb == 3 - sh:
                            pass
                        if pb == 3 and not sh:
                            emit_scans((sp - 1) * CPS, sp * CPS)
                        elif pb == 4 - sh:
                            # Store deferred: mid-stream stores steal
                            # saturated DMA-engine time from the x loads.
                            emit_affine_store((sp - 1) * CPS, sp * CPS,
                                              store=False)
                    # Last span: first half's post stages start as soon as
                    # strips 0-1 finish (after pb3's matmuls).  Scan/affine
                    # arithmetic goes to GpSimd, which is idle once the loads
                    # are issued, keeping Vector free for the PSUM drains.
                    if sp == LAST:
                        if pb == 5:
                            emit_ywmid_h(sp, 0, ps_y)
                        elif pb == 6:
                            emit_backT_h(sp, 0)
                        elif pb == 7:
                            emit_ywcopies_reduce_h(sp, 0)

                if sp < LAST:
                    emit_ywmid(sp, ps_y)
                else:
                    c0 = sp * CPS
                    emit_scans(c0, c0 + 4)
                    emit_affine_store(c0, c0 + 4, eng=nc.gpsimd)
                    emit_ywmid_h(sp, 1, ps_y2)
                    emit_backT_h(sp, 1)
                    emit_ywcopies_reduce_h(sp, 1)
                    # Deferred bulk store of spans 0..6 (one descriptor, on
                    # the Activation HWDGE queue) — transfers run during the
                    # stream wind-down when the DMA engines are idle.
                    nc.scalar.dma_start(
                        out=out_ext[:, 0:LAST * SPAN_T],
                        in_=out_sb[:, 0:LAST * SPAN_T],
                    )
                    emit_scans(c0 + 4, c0 + 8)
                    emit_affine_store(c0 + 4, c0 + 8)

    nc.compile()
    return nc


def kernel(x, quant_bins, change_scales):
    global _cached_nc
    if _cached_nc is None:
        _cached_nc = build_kernel()
    nc = _cached_nc

    x = np.ascontiguousarray(x, dtype=np.float32)
    qb = np.ascontiguousarray(quant_bins, dtype=np.float32).reshape(NB, 1)
    cs = np.ascontiguousarray(change_scales, dtype=np.float32).reshape(NB, 1)

    in_maps = [
        {
            "x": x[i * BS:(i + 1) * BS].reshape(BS, T * NB),
            "quant_bins": qb,
            "change_scales": cs,
        }
        for i in range(NCORES)
    ]
    res = run_bass_kernel_spmd(nc, in_maps, core_ids=list(range(NCORES)))
    out = np.concatenate([res.results[i]["out"] for i in range(NCORES)], axis=0)
    return out.astype(np.float32)


if __name__ == "__main__":
    rng = np.random.default_rng(0)
    x = rng.standard_normal((B, T, NB)).astype(np.float32)
    qb = rng.standard_normal((NB,)).astype(np.float32)
    cs = rng.uniform(0.9, 1.1, (NB, 1)).astype(np.float32)
    out = kernel(x=x, quant_bins=qb, change_scales=cs)
    print("out", out.shape, out.dtype)


# revision 40
# speedup vs baseline: 1.0743x; 1.0743x over previous
"""DeltaDequantization Trainium2 kernel (8-core SPMD, pure data parallel over batch).

Math (per batch element b, chunks c of 32 steps):
    scale_c = (1/32) * sum_{s,n} x[b,c,s,n] * cs[n]          (independent of carry!)
    S_c     = prod_{c'<c} scale_c'          (exclusive cumprod)
    y[b,t]  = sum_n x[b,t,n] * qb[n]
    m_c     = (1/32) * sum_{s in c} y[b,t]
    pred_c  = sum_{c'<c} S_c' * m_c'        (exclusive cumsum)
    out[b,t]= pred_c(t) + S_c(t) * y[b,t]

Kernel: load x naturally [128b, (t,n)], PE-transpose 128x128 blocks to get
(t',n) on partitions, one [128,8] matmul computes y and w=x@cs for 4 t's at a
time, PE-transpose back to [b,t], tensor_tensor_scan for the 64-step
recurrences, affine, store.

Pipelining notes (measured on HW):
- The 16 SWDGE DMA engines run 100% busy during the x stream; per-engine
  HBM-side rate peaks at ~25 B/ns with 8 KiB DRAM rows (quarter-span grain).
  Larger (16K) and smaller (4K) rows are slower.  The stream, not HBM, is
  the bottleneck, so total DMA bytes set an ~85 us floor for the window.
- Engine queues execute in (priority = program) order with head-of-line
  blocking, so span sp's post-matmul stages are EMITTED inside span sp+1's
  pb loop at points where their dependencies are already satisfied.
- The last span's post-processing runs at half-span granularity so only the
  final half-span's chain sits on the serial tail.
"""

import numpy as np
from contextlib import ExitStack

import concourse.bass as bass
import concourse.bacc as bacc
import concourse.tile as tile
from concourse import mybir
from concourse.bass_utils import run_bass_kernel_spmd
from concourse.masks import make_identity

F32 = mybir.dt.float32
BF16 = mybir.dt.bfloat16

B, T, NB = 1024, 2048, 32
NCORES = 8
BS = B // NCORES          # 128 batch rows per core = full partition dim
ADAPT = 32
C = T // ADAPT            # 64 chunks
SPAN_T = 256              # timesteps per pipelined span
NSPAN = T // SPAN_T       # 8
SPAN_F = SPAN_T * NB      # 8192 f32 elements per partition per span
CPS = SPAN_T // ADAPT     # 8 chunks per span

_cached_nc = None


def build_kernel():
    nc = bacc.Bacc("TRN2", target_bir_lowering=False, debug=False)

    x_ext = nc.dram_tensor("x", [BS, T * NB], F32, kind="ExternalInput")
    qb_ext = nc.dram_tensor("quant_bins", [NB, 1], F32, kind="ExternalInput")
    cs_ext = nc.dram_tensor("change_scales", [NB, 1], F32, kind="ExternalInput")
    out_ext = nc.dram_tensor("out", [BS, T], F32, kind="ExternalOutput")

    with tile.TileContext(nc) as tc:
        with (
            tc.tile_pool(name="consts", bufs=1) as consts,
            tc.tile_pool(name="xpool", bufs=10) as xpool,
            tc.tile_pool(name="xtpool", bufs=3) as xtpool,
            tc.tile_pool(name="midpool", bufs=2) as midpool,
            tc.tile_pool(name="accpool", bufs=1) as accpool,
            tc.tile_pool(name="smallpool", bufs=1) as smallpool,
            tc.tile_pool(name="ps_t", bufs=4, space="PSUM") as ps_t,
            tc.tile_pool(name="ps_yw", bufs=2, space="PSUM") as ps_yw,
            tc.tile_pool(name="ps_b", bufs=2, space="PSUM") as ps_b,
        ):
            # Issue the first x cast-loads before anything else touches the
            # GpSimd queue so HBM streaming starts at the preamble's end.
            # Load plan (in pb units of 32 t = 4 KiB f32 rows):
            # quarter-spans (8 KiB rows, the per-engine sweet spot)
            # throughout, split into eighth-spans at the end so the tail's
            # transposes start while the final bytes are in flight.
            PBF = SPAN_F // 8          # f32 elems per pb per partition
            plan = [4] + [2] * 29 + [1, 1]
            assert sum(plan) == NSPAN * 8
            load_pb0 = []
            acc = 0
            for cnt in plan:
                load_pb0.append(acc)
                acc += cnt
            pb2load = {}
            for li, (p0, cnt) in enumerate(zip(load_pb0, plan)):
                for w in range(cnt):
                    pb2load[p0 + w] = (li, w)
            xq = []
            next_issue = [0]
            AHEAD_PB = 16

            def issue_one():
                li = next_issue[0]
                cnt, p0 = plan[li], load_pb0[li]
                x_h = xpool.tile([128, cnt * PBF], BF16)
                nc.gpsimd.dma_start(
                    out=x_h[:], in_=x_ext[:, p0 * PBF:(p0 + cnt) * PBF]
                )
                xq.append(x_h)
                next_issue[0] += 1

            def issue_ahead(global_pb):
                while (
                    next_issue[0] < len(plan)
                    and load_pb0[next_issue[0]] <= global_pb + AHEAD_PB
                ):
                    issue_one()

            issue_one()
            issue_one()

            ident = consts.tile([128, 128], F32)
            make_identity(nc, ident[:])
            ident_bf = consts.tile([128, 128], BF16)
            make_identity(nc, ident_bf[:])

            issue_ahead(-2)  # remaining loads covering pb 0..13

            # Four stationary matrices A32_q [128, 32], q = 0..3.
            # Column m = 16*j + 4*q + t''; A32_q[(t', n), m] = delta(t', t'') *
            # (qb[n] if j == 0 else cs[n]); zero columns for other q values.
            # Stage qb/cs via the Activation HWDGE queue: the Sync queue
            # carries one companion descriptor per SWDGE load.
            qbcs = consts.tile([128, 2], F32)
            for tp in range(4):
                nc.scalar.dma_start(out=qbcs[32 * tp:32 * tp + 32, 0:1], in_=qb_ext[:])
                nc.scalar.dma_start(out=qbcs[32 * tp:32 * tp + 32, 1:2], in_=cs_ext[:])
            A32 = []
            for q in range(4):
                Aq = consts.tile([128, 32], BF16, tag=f"A32_{q}")
                nc.vector.memset(Aq[:], 0.0)
                for tp in range(4):
                    sl = slice(32 * tp, 32 * tp + 32)
                    nc.vector.tensor_copy(
                        out=Aq[sl, 4 * q + tp:4 * q + tp + 1], in_=qbcs[sl, 0:1]
                    )
                    nc.vector.tensor_copy(
                        out=Aq[sl, 16 + 4 * q + tp:16 + 4 * q + tp + 1], in_=qbcs[sl, 1:2]
                    )
                A32.append(Aq)

            # Persistent per-core accumulators
            y_sb = accpool.tile([128, T], F32)
            w_sb = accpool.tile([128, T], F32)
            out_sb = accpool.tile([128, T], F32)

            m_term = smallpool.tile([128, C], F32)
            p_sc = smallpool.tile([128, C], F32)
            S_exc = smallpool.tile([128, C + 1], F32)
            pred = smallpool.tile([128, C + 1], F32)
            tau = smallpool.tile([128, C], F32)
            nc.vector.memset(S_exc[:, 0:1], 1.0)
            nc.vector.memset(pred[:, 0:1], 0.0)

            yw_mid_of = {}
            ps2_of = {}

            def emit_ywmid(sp, ps_y):
                yw_mid = midpool.tile([128, 512], F32)
                nc.scalar.copy(out=yw_mid[:], in_=ps_y[:])
                yw_mid_of[sp] = yw_mid

            def emit_backT(sp):
                yw_mid = yw_mid_of.pop(sp)
                ps2 = ps_b.tile([128, 512], F32, tag="ps2")
                for blk2 in range(4):
                    nc.tensor.transpose(
                        ps2[:, blk2 * 128:(blk2 + 1) * 128],
                        yw_mid[:, blk2 * 128:(blk2 + 1) * 128],
                        ident[:],
                    )
                ps2_of[sp] = ps2

            def emit_ywcopies_reduce(sp):
                # ps2 free index = 128*blk + 32*g4 + 16*j + 4*q + t''
                # t(within span) = 64*g4 + 16*q + 4*blk + t''
                ps2 = ps2_of.pop(sp)
                ps2v = ps2[:].rearrange(
                    "p (b g j q t) -> p g q b j t", b=4, g=4, j=2, q=4, t=4
                )
                yspan = y_sb[:, sp * SPAN_T:(sp + 1) * SPAN_T].rearrange(
                    "p (g q b t) -> p g q b t", g=4, q=4, b=4, t=4
                )
                wspan = w_sb[:, sp * SPAN_T:(sp + 1) * SPAN_T].rearrange(
                    "p (g q b t) -> p g q b t", g=4, q=4, b=4, t=4
                )
                nc.vector.tensor_copy(out=yspan, in_=ps2v[:, :, :, :, 0:1, :].squeeze(4))
                nc.vector.tensor_copy(out=wspan, in_=ps2v[:, :, :, :, 1:2, :].squeeze(4))

                csl = slice(sp * CPS, (sp + 1) * CPS)
                nc.vector.tensor_reduce(
                    out=m_term[:, csl],
                    in_=y_sb[:, sp * SPAN_T:(sp + 1) * SPAN_T].rearrange(
                        "p (c s) -> p c s", c=CPS, s=ADAPT
                    ),
                    axis=mybir.AxisListType.X,
                    op=mybir.AluOpType.add,
                )
                nc.vector.tensor_reduce(
                    out=p_sc[:, csl],
                    in_=w_sb[:, sp * SPAN_T:(sp + 1) * SPAN_T].rearrange(
                        "p (c s) -> p c s", c=CPS, s=ADAPT
                    ),
                    axis=mybir.AxisListType.X,
                    op=mybir.AluOpType.add,
                )

            def emit_scans(c_lo, c_hi, eng=None):
                eng = eng or nc.vector
                sl = slice(c_lo, c_hi)
                eng.tensor_scalar_mul(m_term[:, sl], m_term[:, sl], 1.0 / ADAPT)
                eng.tensor_scalar_mul(p_sc[:, sl], p_sc[:, sl], 1.0 / ADAPT)
                eng.tensor_tensor_scan(
                    out=S_exc[:, c_lo + 1:c_hi + 1],
                    data0=p_sc[:, sl],
                    data1=p_sc[:, sl],
                    initial=S_exc[:, c_lo:c_lo + 1],
                    op0=mybir.AluOpType.mult,
                    op1=mybir.AluOpType.bypass,
                )
                eng.tensor_mul(out=tau[:, sl], in0=S_exc[:, sl], in1=m_term[:, sl])
                eng.tensor_tensor_scan(
                    out=pred[:, c_lo + 1:c_hi + 1],
                    data0=tau[:, sl],
                    data1=tau[:, sl],
                    initial=pred[:, c_lo:c_lo + 1],
                    op0=mybir.AluOpType.add,
                    op1=mybir.AluOpType.bypass,
                )

            def emit_affine_store(c_lo, c_hi, eng=None, store=True):
                eng = eng or nc.vector
                sl = slice(c_lo, c_hi)
                nch = c_hi - c_lo
                t_lo, t_hi = c_lo * ADAPT, c_hi * ADAPT
                y3 = y_sb[:, t_lo:t_hi].rearrange("p (c s) -> p c s", c=nch, s=ADAPT)
                o3 = out_sb[:, t_lo:t_hi].rearrange("p (c s) -> p c s", c=nch, s=ADAPT)
                S_b = S_exc[:, sl].unsqueeze(2).broadcast_to([128, nch, ADAPT])
                pred_b = pred[:, sl].unsqueeze(2).broadcast_to([128, nch, ADAPT])
                eng.tensor_mul(out=o3, in0=y3, in1=S_b)
                eng.tensor_add(out=o3, in0=o3, in1=pred_b)
                if store:
                    nc.sync.dma_start(
                        out=out_ext[:, t_lo:t_hi], in_=out_sb[:, t_lo:t_hi]
                    )

            # ---- half-span post stages (last span only) ----
            ps2h_of = {}

            def emit_ywmid_h(sp, h, src):
                yw_mid = midpool.tile([128, 512], F32)
                nc.scalar.copy(out=yw_mid[64 * h:64 * h + 64, :],
                               in_=src[64 * h:64 * h + 64, :])
                yw_mid_of[(sp, h)] = yw_mid

            def emit_backT_h(sp, h):
                yw_mid = yw_mid_of.pop((sp, h))
                # Full-width tile: shares the ps_b pool's single tag/slots.
                ps2 = ps_b.tile([128, 512], F32, tag="ps2")
                for blk2 in range(4):
                    nc.tensor.transpose(
                        ps2[:, blk2 * 64:(blk2 + 1) * 64],
                        yw_mid[64 * h:64 * h + 64, blk2 * 128:(blk2 + 1) * 128],
                        ident[64 * h:64 * h + 64, 64 * h:64 * h + 64],
                    )
                ps2h_of[(sp, h)] = ps2

            def emit_ywcopies_reduce_h(sp, h):
                # ps2 free index = 64*blk + 32*gl + 16*j + 4*q + t''  (gl = g4-2h)
                # t(within span) = 128*h + 64*gl + 16*q + 4*blk + t''
                ps2 = ps2h_of.pop((sp, h))
                ps2v = ps2[:, 0:256].rearrange(
                    "p (b g j q t) -> p g q b j t", b=4, g=2, j=2, q=4, t=4
                )
                t0 = sp * SPAN_T + 128 * h
                yspan = y_sb[:, t0:t0 + 128].rearrange(
                    "p (g q b t) -> p g q b t", g=2, q=4, b=4, t=4
                )
                wspan = w_sb[:, t0:t0 + 128].rearrange(
                    "p (g q b t) -> p g q b t", g=2, q=4, b=4, t=4
                )
                nc.vector.tensor_copy(out=yspan, in_=ps2v[:, :, :, :, 0:1, :].squeeze(4))
                nc.vector.tensor_copy(out=wspan, in_=ps2v[:, :, :, :, 1:2, :].squeeze(4))

                c0 = sp * CPS + 4 * h
                csl = slice(c0, c0 + 4)
                nc.vector.tensor_reduce(
                    out=m_term[:, csl],
                    in_=y_sb[:, t0:t0 + 128].rearrange(
                        "p (c s) -> p c s", c=4, s=ADAPT
                    ),
                    axis=mybir.AxisListType.X,
                    op=mybir.AluOpType.add,
                )
                nc.vector.tensor_reduce(
                    out=p_sc[:, csl],
                    in_=w_sb[:, t0:t0 + 128].rearrange(
                        "p (c s) -> p c s", c=4, s=ADAPT
                    ),
                    axis=mybir.AxisListType.X,
                    op=mybir.AluOpType.add,
                )

            LAST = NSPAN - 1
            for sp in range(NSPAN):
                xT_sp = xtpool.tile([128, SPAN_F], BF16)
                # y/w projection, interleaved with the transposes that feed it.
                # Group g = g4*4+q covers blocks 4g..4g+3 (t = 16g + 4blk + t'').
                # Strip g4 accumulates 4 matmuls into ps_y[32*g4 : 32*g4+32, :];
                # dense partition layout p = 32*g4 + 16*j + 4*q + t'', free=(blk,b).
                ps_y = ps_yw.tile([128, 512], F32, tag="psy")
                ps_y2 = None
                for pb in range(8):
                    # SWDGE cast-load f32 DRAM -> bf16 SBUF, planned grain
                    global_pb = sp * 8 + pb
                    issue_ahead(global_pb)
                    li, w = pb2load[global_pb]
                    x_h = xq[li]
                    if sp == LAST and pb == 4:
                        # Separate PSUM tile for strips 2-3 so the half-0
                        # yw_mid read doesn't serialize these matmuls
                        # (tile-granular write-after-read hazard).
                        ps_y2 = ps_yw.tile([128, 512], F32, tag="psy")
                    pst = ps_t.tile([128, 1024], BF16)
                    for k in range(8):
                        blk_l = w * 8 + k
                        nc.tensor.transpose(
                            pst[:, k * 128:(k + 1) * 128],
                            x_h[:, blk_l * 128:(blk_l + 1) * 128],
                            ident_bf[:],
                        )
                    dst = xT_sp[:, pb * 1024:(pb + 1) * 1024]
                    # Last span: keep the Scalar queue free for the yw_mid
                    # copies that gate the tail chain.
                    if pb % 2 == 0:
                        nc.vector.tensor_copy(out=dst, in_=pst[:])
                    else:
                        nc.scalar.copy(out=dst, in_=pst[:])
                    for gg in range(2):
                        g = pb * 2 + gg
                        g4, q = divmod(g, 4)
                        tgt = ps_y2 if (sp == LAST and g4 >= 2) else ps_y
                        nc.tensor.matmul(
                            tgt[32 * g4:32 * g4 + 32, :],
                            A32[q][:],
                            xT_sp[:, g * 512:(g + 1) * 512],
                            start=(q == 0),
                            stop=(q == 3),
                            tile_position=(0, 32 * g4),
                        )
                    # Previous span's post stages, placed where their deps are
                    # already met so they never head-of-line-block this span.
                    if sp > 0:
                        # In the last span, run the previous span's scan and
                        # affine one pb earlier so they clear the Vector
                        # queue before the tail's PSUM-drain copies arrive.
                        sh = 1 if sp == LAST else 0
                        if pb == 1:
                            emit_backT(sp - 1)
                        elif pb == 2:
                            emit_ywcopies_reduce(sp - 1)
                            if sh:
                                emit_scans((sp - 1) * CPS, sp * CPS)
                        elif pb == 3 - sh + 0 and pb >= 3 - sh and pb == 3 - sh:
                            pass
                        if pb == 3 and not sh:
                            emit_scans((sp - 1) * CPS, sp * CPS)
                        elif pb == 4 - sh:
                            # Store deferred: mid-stream stores steal
                            # saturated DMA-engine time from the x loads.
                            emit_affine_store((sp - 1) * CPS, sp * CPS,
                                              store=False)
                    # Last span: first half's post stages start as soon as
                    # strips 0-1 finish (after pb3's matmuls).  Scan/affine
                    # arithmetic goes to GpSimd, which is idle once the loads
                    # are issued, keeping Vector free for the PSUM drains.
                    if sp == LAST:
                        if pb == 5:
                            emit_ywmid_h(sp, 0, ps_y)
                        elif pb == 6:
                            emit_backT_h(sp, 0)
                        elif pb == 7:
                            emit_ywcopies_reduce_h(sp, 0)

                if sp < LAST:
                    emit_ywmid(sp, ps_y)
                else:
                    c0 = sp * CPS
                    emit_scans(c0, c0 + 4)
                    emit_affine_store(c0, c0 + 4, eng=nc.gpsimd)
                    emit_ywmid_h(sp, 1, ps_y2)
                    emit_backT_h(sp, 1)
                    emit_ywcopies_reduce_h(sp, 1)
                    # Deferred bulk store of spans 0..6 (one descriptor, on
                    # the Activation HWDGE queue) — transfers run during the
                    # stream wind-down when the DMA engines are idle.
                    nc.scalar.dma_start(
                        out=out_ext[:, 0:LAST * SPAN_T],
                        in_=out_sb[:, 0:LAST * SPAN_T],
                    )
                    emit_scans(c0 + 4, c0 + 8)
                    emit_affine_store(c0 + 4, c0 + 8)

    nc.compile()
    return nc


def kernel(x, quant_bins, change_scales):
    global _cached_nc
    if _cached_nc is None:
        _cached_nc = build_kernel()
    nc = _cached_nc

    x = np.ascontiguousarray(x, dtype=np.float32)
    qb = np.ascontiguousarray(quant_bins, dtype=np.float32).reshape(NB, 1)
    cs = np.ascontiguousarray(change_scales, dtype=np.float32).reshape(NB, 1)

    in_maps = [
        {
            "x": x[i * BS:(i + 1) * BS].reshape(BS, T * NB),
            "quant_bins": qb,
            "change_scales": cs,
        }
        for i in range(NCORES)
    ]
    res = run_bass_kernel_spmd(nc, in_maps, core_ids=list(range(NCORES)))
    out = np.concatenate([res.results[i]["out"] for i in range(NCORES)], axis=0)
    return out.astype(np.float32)


if __name__ == "__main__":
    rng = np.random.default_rng(0)
    x = rng.standard_normal((B, T, NB)).astype(np.float32)
    qb = rng.standard_normal((NB,)).astype(np.float32)
    cs = rng.uniform(0.9, 1.1, (NB, 1)).astype(np.float32)
    out = kernel(x=x, quant_bins=qb, change_scales=cs)
    print("out", out.shape, out.dtype)
